# revision 17
# baseline (speedup 1.0000x reference)
"""Trainium2 Bass kernel for nn_Encoder (NRI-style GNN message-passing encoder).

Reference math:
  h  = MLP1(x)                       [B,N,H]   N=64 nodes, H=128
  e  = MLP2(node2edge(h))            [B,E,H]   E=4032 edges (fully connected)
  n  = MLP3(edge2node(e))            [B,N,H]
  e2 = MLP4([node2edge(n), e])       [B,E,H]
  out= e2 @ wout + bout              [B,E,16]

Distribution: data-parallel over batch, 8 items per core x 8 cores.

v2 design (from v1 + trace analysis; v1 measured 138.7us with PE 74% /
ACT 60% / DVE 61% busy and GpSimd idle):
- Same algebraic restructuring as v1 (edge reorder so node2edge folds
  into matmul access patterns; edge2node as a strided reduce; x_skip and
  the output linear folded host-side).
- GpSimd (Pool) engine recruited for SBUF-only work: hT9/nT9
  replication and the middle levels of the edge2node add-tree.
- Evacuations balanced across ACT/DVE so each engine stays below PE
  (~9.5us/item): ACT ~7.5, DVE ~7.3, Pool ~6.7.
- Software pipeline with explicit per-engine FIFO emission order
  (engines execute in order; a waiting op blocks everything behind it):
  slot s runs stageA(s)=MLP2, stageB(s-1)=tree tail+node chain,
  stageC1(s-2)=MLP4, stageC2(s-3)=out layer + DMA, instructions
  interleaved so every dep is ready (or nearly) when its op reaches the
  head of its engine queue.

The harness calls kernel(**inputs) with full unsharded inputs.
"""
import sys

sys.path.insert(0, "/opt/trn_rl_repo")

import numpy as np

import concourse.bass as bass
from concourse import bacc
import concourse.mybir as mybir
import concourse.tile as tile
from concourse.bass_utils import run_bass_kernel_spmd

F32 = mybir.dt.float32
BF16 = mybir.dt.bfloat16

N_NODES = 64
N_EDGES = 4032
BATCH = 64
N_IN = 64
H = 128
N_OUT = 16
N_CORES = 8
B_LOC = BATCH // N_CORES

# 63 s-blocks (s = 1..63; block s holds edges (sender=(j+s)%64, receiver=j))
# -> 8 uniform chunks of 8 blocks; the last chunk starts at s=56, overlapping
# chunk 6 by one block so every chunk is exactly 512 columns.
CHUNK_D0 = [1, 9, 17, 25, 33, 41, 49, 56]

# wpack column layout (MLP1 weights + biases first, then xT, so the
# prologue can start matmuls while the bulk of the DMA/cast continues)
W1A = 0
W1B = 128
BIA = 256          # 7 bias columns: b1a b1b b2a b2n b3a b3b b4a'
XT0 = 263
W2S = 775
W2R = 903
W2B = 1031
W3A = 1159
W3B = 1287
W4S = 1415
W4R = 1543
W2BK = 1671
W4O = 1799         # [128, 32], cols 16:32 zero padding
WTOT = 1831

_AF = mybir.ActivationFunctionType
_ALU = mybir.AluOpType


def _edge_perm():
    """perm[p] = original edge index of reordered edge p = (s-1)*64 + j,
    which is edge (sender=(j+s)%64, receiver=j)."""
    s, j = np.meshgrid(np.arange(1, 64), np.arange(64), indexing="ij")
    i = (j + s) % 64
    return (i * 63 + (j - (j > i))).reshape(-1)


def _ap(t, off, dims):
    return bass.AP(tensor=t.tensor, offset=t.offset + off, ap=[t.ap[0]] + dims)


def build_kernel():
    nc = bacc.Bacc("TRN2", target_bir_lowering=False, debug=False)

    wpack_d = nc.dram_tensor("wpack", [H, WTOT], F32, kind="ExternalInput").ap()
    y_d = nc.dram_tensor("y", [B_LOC, N_OUT, N_EDGES], F32,
                         kind="ExternalOutput").ap()

    scale2n = 1.0 / (63.0 + 1e-6)

    with tile.TileContext(nc) as tc:
        with (
            tc.tile_pool(name="wp", bufs=1) as wp,
            tc.tile_pool(name="h9p", bufs=2) as h9p,
            tc.tile_pool(name="n9p", bufs=2) as n9p,
            tc.tile_pool(name="h2p", bufs=3) as h2p,
            tc.tile_pool(name="tp", bufs=2) as tp,
            tc.tile_pool(name="h4p", bufs=2) as h4p,
            tc.tile_pool(name="smp", bufs=2) as smp,
            tc.tile_pool(name="op", bufs=2) as op,
            tc.tile_pool(name="pbig", bufs=2, space="PSUM") as pbig,   # [128,1024] x2
            tc.tile_pool(name="ppo", bufs=3, space="PSUM") as ppo,     # [16,512] x3
            tc.tile_pool(name="psml", bufs=1, space="PSUM") as psml,
        ):
            # ---------------- prologue ----------------
            wraw = wp.tile([H, WTOT], F32, tag="wraw")
            nc.sync.dma_start(wraw[:, 0:XT0], wpack_d[:, 0:XT0])
            nc.sync.dma_start(wraw[:, XT0:XT0 + 64], wpack_d[:, XT0:XT0 + 64])
            nc.sync.dma_start(wraw[:, XT0 + 64:W2S], wpack_d[:, XT0 + 64:W2S])
            nc.sync.dma_start(wraw[:, W2S:WTOT], wpack_d[:, W2S:WTOT])
            wall = wp.tile([H, WTOT], BF16, tag="wall")
            # bf16 casts staged to unblock MLP1's first 64 tokens quickly
            nc.vector.tensor_copy(wall[:, 0:BIA], wraw[:, 0:BIA])
            nc.vector.tensor_copy(wall[:, XT0:XT0 + 64], wraw[:, XT0:XT0 + 64])
            nc.vector.tensor_copy(wall[:, XT0 + 64:W2S], wraw[:, XT0 + 64:W2S])
            nc.vector.tensor_copy(wall[:, W2S:WTOT], wraw[:, W2S:WTOT])

            xT = wall[0:N_IN, XT0:XT0 + B_LOC * N_NODES]
            w1a = wall[0:N_IN, W1A:W1A + H]
            w1b = wall[:, W1B:W1B + H]
            w2s = wall[:, W2S:W2S + H]
            w2r = wall[:, W2R:W2R + H]
            w2b = wall[:, W2B:W2B + H]
            w3a = wall[:, W3A:W3A + H]
            w3b = wall[:, W3B:W3B + H]
            w4s = wall[:, W4S:W4S + H]
            w4r = wall[:, W4R:W4R + H]
            w2bk = wall[:, W2BK:W2BK + H]
            w4o = wall[:, W4O:W4O + N_OUT]
            bias = {n: wraw[:, BIA + i:BIA + i + 1]
                    for i, n in enumerate(
                        ["b1a", "b1b", "b2a", "b2n", "b3a", "b3b", "b4a"])}

            # MLP1: item 0's 64 tokens first (unblocks MM2(0)), then rest
            h1T = wp.tile([H, B_LOC * N_NODES], BF16, tag="h1T")
            hT = wp.tile([H, B_LOC * N_NODES], BF16, tag="hT")
            for lo, hi in ((0, N_NODES), (N_NODES, B_LOC * N_NODES)):
                p1 = psml.tile([H, hi - lo], F32, tag="psml")
                nc.tensor.matmul(p1, w1a, xT[:, lo:hi], start=True, stop=True)
                nc.scalar.activation(h1T[:, lo:hi], p1, _AF.Relu,
                                     bias=bias["b1a"])
                p2 = psml.tile([H, hi - lo], F32, tag="psml")
                nc.tensor.matmul(p2, w1b, h1T[:, lo:hi],
                                 start=True, stop=True)
                nc.scalar.activation(hT[:, lo:hi], p2, _AF.Identity,
                                     bias=bias["b1b"])

            # per-item live tiles
            hT9s, h2Ts, As, nT9s, pchs, h4s, outPs = ({}, {}, {}, {},
                                                      {}, {}, {})

            def sender_ap(t2, d0):
                # overlapping windows: block k reads cols d0+k .. d0+k+63
                return _ap(t2, d0, [[1, 8], [1, N_NODES]])

            def recv_ap(t2):
                # the same 64 cols streamed 8x (stride-0 outer dim)
                return _ap(t2, 0, [[0, 8], [1, N_NODES]])

            def mm2_group(a, p):
                """MLP2 psum group p (chunks 2p, 2p+1) for item a."""
                hT9 = hT9s[a]
                pmm = pbig.tile([H, 1024], F32, tag="pbig")
                for k in (0, 1):
                    d0 = CHUNK_D0[2 * p + k]
                    sl = slice(512 * k, 512 * (k + 1))
                    nc.tensor.matmul(pmm[:, sl], w2s, sender_ap(hT9, d0),
                                     start=True, stop=False)
                    nc.tensor.matmul(pmm[:, sl], w2r, recv_ap(hT9),
                                     start=False, stop=True)
                return pmm

            def mm4_group(c, p):
                """MLP4 psum group p for item c."""
                nT9, h2T = nT9s[c], h2Ts[c]
                pm4 = pbig.tile([H, 1024], F32, tag="pbig")
                for k in (0, 1):
                    d0 = CHUNK_D0[2 * p + k]
                    e0 = (d0 - 1) * N_NODES
                    sl = slice(512 * k, 512 * (k + 1))
                    nc.tensor.matmul(pm4[:, sl], w4s, sender_ap(nT9, d0),
                                     start=True, stop=False)
                    nc.tensor.matmul(pm4[:, sl], w4r, recv_ap(nT9),
                                     start=False, stop=False)
                    nc.tensor.matmul(pm4[:, sl], w2bk, h2T[:, e0:e0 + 512],
                                     start=False, stop=True)
                return pm4

            # prefetch hT2 for item 0
            hT9s[0] = h9p.tile([H, 2 * N_NODES], BF16, tag="hT2", name="hT2")
            nc.vector.tensor_copy(
                hT9s[0], _ap(hT, 0, [[0, 2], [1, N_NODES]]))

            for s in range(B_LOC + 2):
                a = s if s < B_LOC else None              # stage A: MLP2
                b1 = s - 1 if 0 <= s - 1 < B_LOC else None  # stage B: tree+chain
                c1 = s - 2 if 0 <= s - 2 < B_LOC else None  # stage C: MLP4+out

                # -- nT2 second half for c1 (its [0:64] was written at
                # the end of the previous slot); needed by mm4 this slot.
                if c1 is not None:
                    nT9 = nT9s[c1]
                    nc.vector.tensor_copy(nT9[:, N_NODES:],
                                          nT9[:, 0:N_NODES])
                # -- Pool: prefetch hT9 for next slot's stage A
                if a is not None and a + 1 < B_LOC:
                    hT9s[a + 1] = h9p.tile([H, 2 * N_NODES], BF16, tag="hT2",
                                           name="hT2")
                    nc.vector.tensor_copy(
                        hT9s[a + 1], _ap(hT, (a + 1) * N_NODES,
                                         [[0, 2], [1, N_NODES]]))

                if c1 is not None:
                    outP = op.tile([N_OUT, N_EDGES], F32, tag="outP")
                    outPs[c1] = outP
                if a is not None:
                    h2Ts[a] = h2p.tile([H, N_EDGES], BF16, tag="h2T", name="h2T")

                def out_pair(h4T, p):
                    # w4o for chunks 2p, 2p+1 straight off the fresh h4T tile
                    for c in (2 * p, 2 * p + 1):
                        hsl = slice(512 * (c % 2), 512 * (c % 2) + 512)
                        po = ppo.tile([N_OUT, 512], F32, tag="ppo")
                        nc.tensor.matmul(po, w4o, h4T[:, hsl],
                                         start=True, stop=True)
                        s0 = 512 * c if c < 7 else 3520
                        if c % 2:
                            nc.vector.tensor_copy(outPs[c1][:, s0:s0 + 512],
                                                  po)
                        else:
                            nc.scalar.activation(outPs[c1][:, s0:s0 + 512],
                                                 po, _AF.Copy)

                if a is not None:
                    pg0 = mm2_group(a, 0)
                    nc.scalar.activation(h2Ts[a][:, 0:1024], pg0, _AF.Relu,
                                         bias=bias["b2a"])
                    pg1 = mm2_group(a, 1)
                    nc.vector.tensor_scalar(h2Ts[a][:, 1024:2048], pg1,
                                            bias["b2a"], 0.0, _ALU.add,
                                            _ALU.max)

                # MLP4 p0 while stage-A continues
                if c1 is not None:
                    h4s[c1] = []
                    pm = mm4_group(c1, 0)
                    h4T = h4p.tile([H, 1024], BF16, tag="h4_0")
                    nc.scalar.activation(h4T, pm, _AF.Relu, bias=bias["b4a"])
                    h4s[c1].append(h4T)
                    out_pair(h4T, 0)

                if a is not None:
                    pg2 = mm2_group(a, 2)
                    nc.scalar.activation(h2Ts[a][:, 2048:3072], pg2, _AF.Relu,
                                         bias=bias["b2a"])
                    pg3 = mm2_group(a, 3)
                    # chunk 7 overlaps chunk 6 by one block: write cols
                    # 3072:3584 from psum[0:512] and 3584:4032 from
                    # psum[576:1024] (no overlapping writes).
                    nc.scalar.activation(h2Ts[a][:, 3072:3584], pg3[:, 0:512],
                                         _AF.Relu, bias=bias["b2a"])
                    nc.vector.tensor_scalar(h2Ts[a][:, 3584:4032],
                                            pg3[:, 576:1024], bias["b2a"],
                                            0.0, _ALU.add, _ALU.max)

                # node chain for b1, spread between MLP4 groups below
                if b1 is not None:
                    A = As[b1]
                    pagg = psml.tile([H, N_NODES], F32, tag="psml")
                    nc.tensor.matmul(pagg, w2b, A[:, 0:64],
                                     start=True, stop=True)
                    aggT = smp.tile([H, N_NODES], BF16, tag="aggT")
                    nc.vector.tensor_scalar(aggT, pagg, scale2n,
                                            bias["b2n"], _ALU.mult, _ALU.add)

                if c1 is not None:
                    pm = mm4_group(c1, 1)
                    h4T = h4p.tile([H, 1024], BF16, tag="h4_1")
                    nc.vector.tensor_scalar(h4T, pm, bias["b4a"], 0.0,
                                            _ALU.add, _ALU.max)
                    h4s[c1].append(h4T)
                    out_pair(h4T, 1)

                if b1 is not None:
                    pn1 = psml.tile([H, N_NODES], F32, tag="psml")
                    nc.tensor.matmul(pn1, w3a, aggT, start=True, stop=True)
                    n1T = smp.tile([H, N_NODES], BF16, tag="n1T")
                    nc.vector.tensor_scalar(n1T, pn1, bias["b3a"],
                                            0.0, _ALU.add, _ALU.max)

                if c1 is not None:
                    pm = mm4_group(c1, 2)
                    h4T = h4p.tile([H, 1024], BF16, tag="h4_2")
                    nc.scalar.activation(h4T, pm, _AF.Relu, bias=bias["b4a"])
                    h4s[c1].append(h4T)
                    out_pair(h4T, 2)

                if b1 is not None:
                    nT9s[b1] = n9p.tile([H, 2 * N_NODES], BF16, tag="nT2",
                                        name="nT2")
                    pn2 = psml.tile([H, N_NODES], F32, tag="psml")
                    pchs[b1] = pn2
                    nc.tensor.matmul(pn2, w3b, n1T, start=True, stop=True)

                if c1 is not None:
                    pm = mm4_group(c1, 3)
                    h4T = h4p.tile([H, 1024], BF16, tag="h4_3")
                    nc.vector.tensor_scalar(h4T, pm, bias["b4a"], 0.0,
                                            _ALU.add, _ALU.max)
                    h4s[c1].append(h4T)
                    out_pair(h4T, 3)
                    nc.sync.dma_start(y_d[c1], outPs[c1])

                if b1 is not None:
                    nc.scalar.activation(nT9s[b1][:, 0:N_NODES], pchs[b1],
                                         _AF.Identity, bias=bias["b3b"])

                # edge2node add-tree for a, entirely on the otherwise-idle
                # Pool engine (slow but off the critical path: the chain that
                # consumes S runs mid-way through the NEXT slot).
                if a is not None:
                    A = tp.tile([H, 2048], BF16, tag="A")
                    As[a] = A
                    nc.gpsimd.tensor_add(A[:, 0:1984], h2Ts[a][:, 0:1984],
                                         h2Ts[a][:, 2048:4032])
                    nc.gpsimd.tensor_copy(A[:, 1984:2048],
                                          h2Ts[a][:, 1984:2048])
                    for w in (1024, 512, 256, 128, 64):
                        nc.gpsimd.tensor_add(A[:, 0:w], A[:, 0:w],
                                             A[:, w:2 * w])

    nc.compile()
    return nc


_CACHE = {}


def _get_nc():
    if "nc" not in _CACHE:
        _CACHE["nc"] = build_kernel()
        _CACHE["perm"] = _edge_perm()
    return _CACHE["nc"], _CACHE["perm"]


def make_in_maps(inputs):
    f = lambda k: np.asarray(inputs[k], np.float32)
    w2b = f("w2b")
    w4a = f("w4a")
    b2b = f("b2b")
    w4a_k = w4a[2 * H:]

    def pad128(a):
        out = np.zeros((H, a.shape[1]), np.float32)
        out[:a.shape[0]] = a
        return out

    wfix = np.zeros((H, WTOT), np.float32)

    def put(col, blk):
        wfix[:, col:col + blk.shape[1]] = blk

    put(W1A, pad128(f("w1a")))
    put(W1B, f("w1b"))
    put(W2S, f("w2a")[:H])
    put(W2R, f("w2a")[H:])
    put(W2B, w2b)
    put(W3A, f("w3a"))
    put(W3B, f("w3b"))
    put(W4S, w4a[:H])
    put(W4R, w4a[H:2 * H])
    put(W2BK, w2b @ w4a_k)
    put(W4O, np.pad(f("w4b") @ f("wout"), ((0, 0), (0, 16))))
    bcols = [f("b1a"), f("b1b"), f("b2a"),
             63.0 * b2b / (63.0 + 1e-6), f("b3a"), f("b3b"),
             f("b4a") + b2b @ w4a_k]
    for i, v in enumerate(bcols):
        wfix[:, BIA + i] = v

    x = np.ascontiguousarray(inputs["x"], np.float32)
    in_maps = []
    for c in range(N_CORES):
        xs = x[c * B_LOC:(c + 1) * B_LOC]
        w = wfix.copy()
        w[:, XT0:XT0 + B_LOC * N_NODES] = pad128(
            xs.reshape(B_LOC * N_NODES, N_IN).T)
        in_maps.append({"wpack": np.ascontiguousarray(w)})
    return in_maps


def gather_out(results, perm, inputs):
    b4o = (np.asarray(inputs["b4b"], np.float32) @ inputs["wout"]
           + inputs["bout"]).astype(np.float32)  # [16]
    inv = np.empty_like(perm)
    inv[perm] = np.arange(N_EDGES)
    out = np.empty((BATCH, N_EDGES, N_OUT), np.float32)
    for cr in range(N_CORES):
        y = np.asarray(results[cr]["y"])  # [B_LOC, 16, 4032]
        full = y.transpose(0, 2, 1) + b4o  # [B_LOC, E'', 16]
        out[cr * B_LOC:(cr + 1) * B_LOC] = full[:, inv, :]
    return out


def kernel(**inputs):
    nc, perm = _get_nc()
    in_maps = make_in_maps(inputs)
    res = run_bass_kernel_spmd(nc, in_maps, core_ids=list(range(N_CORES)))
    return gather_out(res.results, perm, inputs)
